# revision 1
# baseline (speedup 1.0000x reference)
"""Trainium2 Bass kernel for nn_Graphs (soft decision-graph probability propagation).

Reference math (G=4 graphs, B=128 batch, N=255 internal nodes, L=256 leaves,
F=512 features, J=8 jumps):
  b  = sigmoid(x @ W_g^T + bias_g)                  (per graph: B x N)
  M0 = softmax(M_left, axis=dest), M1 = softmax(M_right, axis=dest)
  q  = [b*(M1-M0)+M0 | leaf-identity]               (per (g,batch): 511x511)
  prob <- q @ prob, J times, starting from e0; return leaf probs.

Key restructure: q is never materialized. With u = prob[internal] and
v = b * u, one jump is
  prob_new = M0 @ u + (M1-M0) @ v   (+ leaf mass preserved)
where M0/Md are per-graph (511,255) matrices -> two small matmuls per jump.
Leaf rows only ever accumulate, so they live in a persistent PSUM
accumulator across all 8 jumps.

Sharding: 8 cores = (graph g = core//2) x (batch half h = core%2, 64 rows).
No cross-core communication. Host pre-transposes/pads inputs so the device
does zero transposes:
  - mlt/mrt (256,512): M^T with source node on partitions (pad row 255 = 0)
    and destination on free dim, leaf destinations shifted to cols 256..511
    (col 255 = -1e4 pad -> exp = 0). Softmax over dest = free-dim reduce.
  - wt (512,256): W_g^T (feature on partitions), pad node col 255 = 0.
  - xt (512,64): x_half^T.
  - biasp (256,1): bias_g padded.
Output per core: (256,64) leaf-major; host transposes/assembles to (B,L,G)
and applies the reference interval clamp.
"""

import numpy as np

G, B, N, L, F, J = 4, 128, 255, 256, 512, 8
BH = B // 2  # 64 batch rows per core
NCORES = 8
NEG = np.float32(-1e4)

_CACHE = {}


def _build_program():
    import concourse.mybir as mybir
    from concourse import bacc
    from concourse.tile import TileContext

    f32 = mybir.dt.float32
    bf16 = mybir.dt.bfloat16
    f32r = mybir.dt.float32r  # single-pass fp32 matmul mode (4x faster at N>=256)
    AF = mybir.ActivationFunctionType
    AX = mybir.AxisListType

    def rmm(out, lhsT, rhs, **kw):
        nc.tensor.matmul(out, lhsT, rhs, **kw)

    # Bacc (not raw Bass): its compile() pass splits multi-wait instructions
    # into event semaphores, which the TRN2 ISA requires (1 wait/inst max).
    nc = bacc.Bacc(None)
    p_mlt = nc.declare_dram_parameter("mlt", [256, 512], f32, isOutput=False)
    p_mrt = nc.declare_dram_parameter("mrt", [256, 512], f32, isOutput=False)
    # wt (512,256) and xt (512,64) packed side by side: one DMA per K-tile so
    # each b-matmul's lhsT and rhs share a single DMA semaphore (the ISA
    # allows only one sync wait on a Matmult's LDWEIGHTS).
    p_wx = nc.declare_dram_parameter("wx", [512, 256 + BH], f32r, isOutput=False)
    p_bias = nc.declare_dram_parameter("biasp", [256, 1], f32, isOutput=False)
    p_out = nc.declare_dram_parameter("out", [BH, 256], f32, isOutput=True)

    with TileContext(nc) as tc:
        with (
            tc.tile_pool(name="consts", bufs=1) as consts,
            tc.tile_pool(name="work", bufs=2) as work,
            tc.tile_pool(name="state", bufs=2) as state,
            tc.tile_pool(name="psum", bufs=2, space="PSUM") as psum,
            tc.tile_pool(name="psum_acc", bufs=1, space="PSUM") as psum_acc,
        ):
            # ---- PE warm-up ----
            # The jump chain is a dense back-to-back matmul stream; at the
            # cold PE clock it runs 2x slow. Feed the otherwise-idle PE a
            # dummy bf16 stream during the DMA/softmax prefix so the clock
            # has ramped before the real work arrives.
            wsc = consts.tile([128, 128], bf16, tag="wsc", name="wsc")
            rsc = consts.tile([128, 512], bf16, tag="rsc", name="rsc")
            nc.vector.memset(wsc[:], 0.0)
            nc.vector.memset(rsc[:], 0.0)
            pwarm = psum_acc.tile([128, 512], f32, tag="pwarm", name="pwarm")

            def warm(n):
                for _ in range(n):
                    nc.tensor.matmul(pwarm[:], wsc[:], rsc[:], start=True, stop=True)

            # ---- load inputs ----
            # The M-matrix path (DMA -> exp -> recip -> c0/c1) is the long
            # pole: issue its DMAs first on the HWDGE (sync) queues, in
            # half-tiles so exp can start on each half as it lands. wx/bias
            # go through gpsimd so their descriptor issue doesn't serialize
            # behind the M DMAs on the sync sequencer.
            eraw = [consts.tile([128, 512], f32, tag=f"eraw{i}", name=f"eraw{i}") for i in range(4)]
            for i, p_m in ((0, p_mlt), (2, p_mrt)):
                for t in range(2):
                    for hh in range(2):
                        nc.sync.dma_start(
                            eraw[i + t][:, hh * 256:(hh + 1) * 256],
                            p_m[t * 128:(t + 1) * 128, hh * 256:(hh + 1) * 256],
                        )
            wx = [consts.tile([128, 256 + BH], f32r, tag=f"wx{k}", name=f"wx{k}") for k in range(4)]
            for k in range(4):
                nc.gpsimd.dma_start(wx[k][:], p_wx[k * 128:(k + 1) * 128, :])
            bias = [consts.tile([128, 1], f32, tag=f"bias{t}", name=f"bias{t}") for t in range(2)]
            for t in range(2):
                nc.gpsimd.dma_start(bias[t][:], p_bias[t * 128:(t + 1) * 128, :])

            # ---- softmax, lazily normalized ----
            # el/er hold raw exp(M^T) (f32r); the softmax denominators r0/r1
            # are folded into the per-jump state scaling instead of scaling
            # the big matrices:  M0n@u + (M1n-M0n)@(b*u)
            #                  = E0@(r0*(1-b)*u) + E1@(r1*b*u)
            # exp runs per half-tile with accum_out computing the row sums
            # inline (no separate reduce).
            el = [consts.tile([128, 512], f32r, tag=f"el{t}", name=f"el{t}") for t in range(2)]
            er = [consts.tile([128, 512], f32r, tag=f"er{t}", name=f"er{t}") for t in range(2)]
            rec = []
            for i, mat in enumerate((el[0], el[1], er[0], er[1])):
                ps = [work.tile([128, 1], f32, tag=f"ps{hh}", name=f"ps{hh}") for hh in range(2)]
                for hh in range(2):
                    sl = slice(hh * 256, (hh + 1) * 256)
                    nc.scalar.activation(mat[:, sl], eraw[i][:, sl], AF.Exp,
                                         accum_out=ps[hh][:])
                s = work.tile([128, 1], f32, tag="ssum", name="ssum")
                nc.vector.tensor_add(s[:], ps[0][:], ps[1][:])
                r = consts.tile([128, 1], f32, tag=f"srec{i}", name=f"srec{i}")
                nc.vector.reciprocal(r[:], s[:])
                rec.append(r)

            # ---- b = sigmoid(W @ x^T + bias) via exp (avoids a second ACT
            # table load), node-major (256,64); then fold softmax denominators:
            # c0 = r0*(1-b) = r0*eb/(1+eb), c1 = r1*b = r1/(1+eb), eb=exp(-logit)
            # c01[t][:,0] = c0 = r0*(1-b), c01[t][:,1] = c1 = r1*b -- packed so
            # the per-jump scaling is a single DVE op per tile half
            c01 = [consts.tile([128, 2, BH], f32, tag=f"c01{t}", name=f"c01{t}") for t in range(2)]
            warm(14)
            for mh in range(2):
                pb = psum.tile([128, BH], f32, tag="pb", name="pb")
                for k in range(4):
                    rmm(
                        pb[:], wx[k][:, mh * 128:(mh + 1) * 128],
                        wx[k][:, 256:256 + BH],
                        start=(k == 0), stop=(k == 3),
                    )
                eb = work.tile([128, BH], f32, tag="eb", name="eb")
                nc.scalar.activation(eb[:], pb[:], AF.Exp, bias=bias[mh][:], scale=-1.0)
                den = work.tile([128, BH], f32, tag="den", name="den")
                nc.vector.tensor_scalar_add(den[:], eb[:], 1.0)
                sig = work.tile([128, BH], f32, tag="sig", name="sig")
                nc.vector.reciprocal(sig[:], den[:])
                nc.vector.tensor_scalar_mul(c01[mh][:, 1], sig[:], rec[2 + mh][:])
                nc.vector.tensor_mul(sig[:], sig[:], eb[:])
                nc.vector.tensor_scalar_mul(c01[mh][:, 0], sig[:], rec[mh][:])
            warm(12)

            # ---- jump loop ----
            # State is kept only as the scaled pair (up = c0*u, v = c1*u),
            # computed by DVE directly from the previous jump's PSUM -- no
            # intermediate u copy. Jump 0 seeds from a one-hot z (f32).
            z = [state.tile([128, BH], f32, tag=f"z{t}", name=f"z{t}") for t in range(2)]
            nc.vector.memset(z[0][:], 0.0)
            nc.vector.memset(z[1][:], 0.0)
            nc.vector.memset(z[0][0:1, :], 1.0)
            # leaf accumulator: batch-major (64,256), one PSUM bank, N=256
            # moving dim keeps fp32r at 1 cycle/row.
            pleaf = psum_acc.tile([BH, 256], f32, tag="pl", name="pl")

            mult = mybir.AluOpType.mult
            pq = None
            for j in range(J):
                # upv[t][:,0] = c0*u, upv[t][:,1] = c1*u -- one DVE op per
                # tile half, reading the previous jump's PSUM via broadcast
                upv = [state.tile([128, 2, BH], f32r, tag=f"upv{t}", name=f"upv{t}") for t in range(2)]
                for t in range(2):
                    s = z[t] if j == 0 else pq[t]
                    nc.vector.tensor_tensor(
                        out=upv[t][:], in0=c01[t][:],
                        in1=s[:, None, :].broadcast_to([128, 2, BH]), op=mult)
                up = [upv[t][:, 0] for t in range(2)]
                v = [upv[t][:, 1] for t in range(2)]
                # internal-node block: node-major, fresh psum per jump.
                # Skipped on the last jump (u_J is never read).
                if j < J - 1:
                    pq = [psum.tile([128, BH], f32, tag=f"pq{mt}", name=f"pq{mt}") for mt in range(2)]
                    for mt in range(2):
                        ms = slice(mt * 128, (mt + 1) * 128)
                        rmm(pq[mt][:], el[0][:, ms], up[0], start=True, stop=False)
                        rmm(pq[mt][:], el[1][:, ms], up[1], start=False, stop=False)
                        rmm(pq[mt][:], er[0][:, ms], v[0], start=False, stop=False)
                        rmm(pq[mt][:], er[1][:, ms], v[1], start=False, stop=True)
                # leaf block: batch-major (lhsT = state), accumulated in PSUM
                # across all jumps; also covers the DVE latency of the next
                # jump's upv computation so the PE stream stays gapless
                first = j == 0
                last = j == J - 1
                rmm(pleaf[:], up[0], el[0][:, 256:512], start=first, stop=False)
                rmm(pleaf[:], up[1], el[1][:, 256:512], start=False, stop=False)
                rmm(pleaf[:], v[0], er[0][:, 256:512], start=False, stop=False)
                rmm(pleaf[:], v[1], er[1][:, 256:512], start=False, stop=last)

            # ---- output ----
            o = work.tile([BH, 256], f32, tag="o", name="o")
            nc.vector.tensor_copy(o[:], pleaf[:])
            nc.sync.dma_start(p_out[:, :], o[:])

    nc.finalize()
    return nc


def _get_program():
    if "nc" not in _CACHE:
        _CACHE["nc"] = _build_program()
    return _CACHE["nc"]


def _prep_inputs(x, W, bias, M_left, M_right):
    """Host-side shard + layout prep. Core c -> graph c//2, batch half c%2."""
    in_maps = []
    mlt_g, mrt_g, wt_g, bias_g = [], [], [], []
    for g in range(G):
        mlt = np.zeros((256, 512), np.float32)
        mrt = np.zeros((256, 512), np.float32)
        tl = np.ascontiguousarray(M_left[g].T)   # (255, 511)
        tr = np.ascontiguousarray(M_right[g].T)
        for dst, src in ((mlt, tl), (mrt, tr)):
            dst[0:255, 0:255] = src[:, 0:255]
            dst[0:255, 256:512] = src[:, 255:511]
            dst[0:255, 255] = NEG
        mlt_g.append(mlt)
        mrt_g.append(mrt)
        wt = np.zeros((512, 256), np.float32)
        wt[:, 0:255] = W[g].T
        wt_g.append(wt)
        # negated: the device computes exp(-(logit)) as exp(psum*-1 + bias_ap),
        # so bias_ap must carry -bias
        bp = np.zeros((256, 1), np.float32)
        bp[0:255, 0] = -bias[g]
        bias_g.append(bp)
    xt_h = [np.ascontiguousarray(x[h * BH:(h + 1) * BH].T) for h in range(2)]
    for c in range(NCORES):
        g, h = c // 2, c % 2
        wx = np.ascontiguousarray(np.concatenate([wt_g[g], xt_h[h]], axis=1))
        in_maps.append({
            "mlt": mlt_g[g], "mrt": mrt_g[g], "wx": wx, "biasp": bias_g[g],
        })
    return in_maps


def _assemble(results):
    eps = np.float32(1e-5)
    ret = np.empty((B, L, G), np.float32)
    for c in range(NCORES):
        g, h = c // 2, c % 2
        ret[h * BH:(h + 1) * BH, :, g] = results[c]["out"]
    ret = np.where(ret > 0.0, ret, eps)
    ret = np.where(ret < 1.0, ret, np.float32(1.0) - eps)
    return ret.astype(np.float32)


def run_on_device(in_maps, trace=False, **kw):
    from concourse.bass_utils import run_bass_kernel_spmd
    nc = _get_program()
    return run_bass_kernel_spmd(nc, in_maps, list(range(NCORES)), trace=trace, **kw)


def kernel(x, W, bias, M_left, M_right):
    in_maps = _prep_inputs(
        np.asarray(x, np.float32), np.asarray(W, np.float32),
        np.asarray(bias, np.float32), np.asarray(M_left, np.float32),
        np.asarray(M_right, np.float32),
    )
    res = run_on_device(in_maps)
    return _assemble(res.results)



# revision 7
# speedup vs baseline: 1.3041x; 1.3041x over previous
"""Trainium2 Bass kernel for nn_Graphs (soft decision-graph probability propagation).

Reference math (G=4 graphs, B=128 batch, N=255 internal nodes, L=256 leaves,
F=512 features, J=8 jumps):
  b  = sigmoid(x @ W_g^T + bias_g)                  (per graph: B x N)
  M0 = softmax(M_left, axis=dest), M1 = softmax(M_right, axis=dest)
  q  = [b*(M1-M0)+M0 | leaf-identity]               (per (g,batch): 511x511)
  prob <- q @ prob, J times, starting from e0; return leaf probs.

Restructure (v2, all-bf16 datapath):
  - q never materialized. With u = prob[internal], one jump is
      u' = E0 @ (r0*(1-b)*u) + E1 @ (r1*b*u)
    where E0/E1 are raw exp(M^T) tiles (bf16) and the softmax denominators
    r0/r1 are folded into the per-(node,batch) coefficients c0/c1.
  - Leaf rows only accumulate, and c0/c1 are jump-invariant, so the leaf
    block is hoisted out of the loop entirely:
      w = E0_leaf @ (sum_j c0*u_j) + E1_leaf @ (sum_j c1*u_j)
    The running sums (sacc) are maintained by gpsimd adds in the shadow of
    the PE jump stream; 4 leaf matmuls run once at the end.
  - Jump 0 is an outer product (u_0 = e0): 4 contract-dim-1 matmuls reading
    row 0 of E0/E1 against row 0 of the coefficients.
  - exp is one fused 1024-col ACT op per src tile (both matrices at once)
    with accum_out giving the combined row sum; a DVE half-reduce splits it
    into the two softmax denominators (r1 = recip(s01 - s_el)).
  - PE warm-up (HAM un-throttle) runs first and is chained INTO the real
    dependency graph (zj = 0*pwarm feeds the c01 coefficient ops, and two
    warm matmuls WAW-target the b-matmul psum), so the scheduler cannot
    push it to the end of the program (which is what happened in v1).

Sharding: 8 cores = (graph g = core//2) x (batch half h = core%2, 64 rows).
No cross-core communication. Host pre-transposes/pads/casts to bf16:
  - m2 (256,1024) bf16: M^T with source node on partitions; cols [0:512] =
    left matrix, [512:1024] = right; each 512 block = [internal 255 | NEG |
    leaf 256] (NEG pad -> exp = 0).
  - wxp (128,1280) bf16: per F-tile k, cols [320k:320k+256] = W_g^T block,
    [320k+256:320k+320] = x_half^T block.
  - biasp (256,1) f32: -bias (device computes exp(-logit) as exp(-1*psum + bias)).
Output per core: (64,256) f32 leaf-major; host assembles to (B,L,G) and
applies the reference interval clamp.
"""

import numpy as np
import ml_dtypes

G, B, N, L, F, J = 4, 128, 255, 256, 512, 8
BH = B // 2  # 64 batch rows per core
NCORES = 8
NEG = np.float32(-1e4)
BF16 = ml_dtypes.bfloat16

_CACHE = {}


def _build_program():
    import concourse.mybir as mybir
    from concourse import bacc
    from concourse.tile import TileContext

    f32 = mybir.dt.float32
    bf16 = mybir.dt.bfloat16
    AF = mybir.ActivationFunctionType
    AX = mybir.AxisListType
    mult = mybir.AluOpType.mult
    add = mybir.AluOpType.add

    nc = bacc.Bacc(None)
    p_m2 = nc.declare_dram_parameter("m2", [256, 1024], bf16, isOutput=False)
    p_wx = nc.declare_dram_parameter("wxp", [128, 1280], bf16, isOutput=False)
    p_bias = nc.declare_dram_parameter("biasp", [256, 1], f32, isOutput=False)
    p_out = nc.declare_dram_parameter("out", [BH, 256], f32, isOutput=True)

    with TileContext(nc) as tc:
        with (
            tc.tile_pool(name="consts", bufs=1) as consts,
            tc.tile_pool(name="work", bufs=2) as work,
            tc.tile_pool(name="state", bufs=3) as state,
            tc.tile_pool(name="psum", bufs=2, space="PSUM") as psum,
            tc.tile_pool(name="psum_acc", bufs=1, space="PSUM") as psum_acc,
        ):
            # ---- DMA issue (first: these gate everything) ----
            # m2 on HWDGE (sync) in two 256KB chunks so exp of chunk 0 can
            # overlap the chunk-1 transfer; wxp/bias on SWDGE (gpsimd) so
            # their descriptors don't queue behind the m2 transfers.
            ch = [consts.tile([128, 1024], bf16, tag=f"ch{t}", name=f"ch{t}") for t in range(2)]
            for t in range(2):
                nc.sync.dma_start(ch[t][:], p_m2[t * 128:(t + 1) * 128, :])
            wx = consts.tile([128, 1280], bf16, tag="wx", name="wx")
            nc.gpsimd.dma_start(wx[:], p_wx[:, :])
            bias = [consts.tile([128, 1], f32, tag=f"bias{t}", name=f"bias{t}") for t in range(2)]
            for t in range(2):
                nc.gpsimd.dma_start(bias[t][:], p_bias[t * 128:(t + 1) * 128, :])

            # ---- PE pre-touch + scheduling anchor ----
            # Two throwaway matmuls into the b-matmul psum. They (a) WAW-force
            # the real b-matmuls to run after them (so nothing framework-side
            # reorders them to the end, as happened to v1's dead-code warm-up),
            # and (b) feed zj = 0*pb, which is added into the c01 coefficient
            # ops below so the whole chain is anchored in the real graph.
            # (A full HAM warm-up stream isn't worth it here: the jump loop is
            # DVE-bound, so a cold PE doesn't set the cadence.)
            wsc = consts.tile([128, 128], bf16, tag="wsc", name="wsc")
            rsc = consts.tile([128, BH], bf16, tag="rsc", name="rsc")
            nc.vector.memset(wsc[:], 0.0)
            nc.vector.memset(rsc[:], 0.0)
            pb = psum.tile([128, 2, BH], f32, tag="pb", name="pb")
            for mh in range(2):
                nc.tensor.matmul(pb[:, mh, :], wsc[:], rsc[:], start=True, stop=True)
            zj = work.tile([128, BH], f32, tag="zj", name="zj")
            nc.vector.tensor_scalar_mul(zj[:], pb[:, 0, :], 0.0)

            # ---- b = sigmoid(W @ x^T + bias) ----
            for mh in range(2):
                for k in range(4):
                    nc.tensor.matmul(
                        pb[:, mh, :],
                        wx[:, k * 320 + mh * 128:k * 320 + (mh + 1) * 128],
                        wx[:, k * 320 + 256:k * 320 + 320],
                        start=(k == 0), stop=(k == 3),
                    )

            # ---- exp of both matrices ----
            # Four 512-col slice-ops (ACT queue: el0, er0, el1, er1, eb0, eb1)
            # so each starts as soon as its chunk lands and each accum_out
            # yields that matrix-half's softmax row sum directly.
            elr = [consts.tile([128, 1024], bf16, tag=f"elr{t}", name=f"elr{t}") for t in range(2)]
            psl = [work.tile([128, 1], f32, tag=f"psl{t}", name=f"psl{t}") for t in range(2)]
            psr = [work.tile([128, 1], f32, tag=f"psr{t}", name=f"psr{t}") for t in range(2)]
            eb = [work.tile([128, BH], f32, tag=f"eb{mh}", name=f"eb{mh}") for mh in range(2)]
            for t in range(2):
                nc.scalar.activation(elr[t][:, 0:512], ch[t][:, 0:512], AF.Exp,
                                     accum_out=psl[t][:])
                nc.scalar.activation(elr[t][:, 512:1024], ch[t][:, 512:1024], AF.Exp,
                                     accum_out=psr[t][:])
            for mh in range(2):
                nc.scalar.activation(eb[mh][:], pb[:, mh, :], AF.Exp,
                                     bias=bias[mh][:], scale=-1.0)

            # ---- softmax denominators + c01 coefficients (DVE) ----
            # c01[t][:,0,:] = r0*(1-b) = r0*eb/(1+eb), c01[t][:,1,:] = r1*b
            # = r1/(1+eb); r0/r1 = 1/accum sums. The "+ zj" (zero) anchors
            # the pre-touch matmuls in the graph.
            c01 = [consts.tile([128, 2, BH], f32, tag=f"c01{t}", name=f"c01{t}") for t in range(2)]
            c01r0 = state.tile([1, 2, BH], bf16, tag="c01r0", name="c01r0")
            r0 = [work.tile([128, 1], f32, tag=f"r0_{t}", name=f"r0_{t}") for t in range(2)]
            r1 = [work.tile([128, 1], f32, tag=f"r1_{t}", name=f"r1_{t}") for t in range(2)]
            for t in range(2):
                nc.vector.reciprocal(r0[t][:], psl[t][:])
                nc.vector.reciprocal(r1[t][:], psr[t][:])
            for t in range(2):
                den = work.tile([128, BH], f32, tag="den", name=f"den{t}")
                nc.vector.tensor_scalar_add(den[:], eb[t][:], 1.0)
                sig = work.tile([128, BH], f32, tag="sig", name=f"sig{t}")
                nc.vector.reciprocal(sig[:], den[:])
                nc.vector.scalar_tensor_tensor(
                    c01[t][:, 1, :], sig[:], r1[t][:], zj[:], op0=mult, op1=add)
                sigeb = work.tile([128, BH], f32, tag="sigeb", name=f"sigeb{t}")
                nc.vector.tensor_mul(sigeb[:], sig[:], eb[t][:])
                nc.vector.scalar_tensor_tensor(
                    c01[t][:, 0, :], sigeb[:], r0[t][:], zj[:], op0=mult, op1=add)
                if t == 0:
                    # bf16 seed state for jump 0 (= c01 row 0, since u_0 = e0)
                    nc.vector.tensor_copy(c01r0[:], c01[0][0:1, :, :])

            # ---- leaf-sum accumulators (maintained by gpsimd) ----
            # sacc f32 through jump 6; jump 7's add writes the bf16 copy
            # directly (the cast the leaf matmuls need comes for free).
            sacc = [consts.tile([128, 2, BH], f32, tag=f"sacc{t}", name=f"sacc{t}") for t in range(2)]
            sacc_bf = [state.tile([128, 2, BH], bf16, tag=f"sbf{t}", name=f"sbf{t}") for t in range(2)]
            nc.gpsimd.memset(sacc[0][:], 0.0)
            nc.gpsimd.memset(sacc[1][:], 0.0)
            nc.gpsimd.tensor_add(sacc[0][0:1, :, :], sacc[0][0:1, :, :], c01r0[:])

            # ---- jump loop ----
            # Jump 0: u_1 = outer(E row0, c01 row0) -> 4 contract-1 matmuls.
            # Jumps 1..6: full rounds, 8 matmuls each, grouped by dest tile
            # so the next jump's DVE scale op starts after the first group.
            # Jump 7: scale+accumulate only (u_8 internal mass is unused).
            pq = [psum.tile([128, BH], f32, tag=f"pq{mt}", name=f"pq{mt}") for mt in range(2)]
            for mt in range(2):
                ms = slice(mt * 128, (mt + 1) * 128)
                nc.tensor.matmul(pq[mt][:], elr[0][0:1, ms], c01r0[0:1, 0, :],
                                 start=True, stop=False)
                ms2 = slice(512 + mt * 128, 512 + (mt + 1) * 128)
                nc.tensor.matmul(pq[mt][:], elr[0][0:1, ms2], c01r0[0:1, 1, :],
                                 start=False, stop=True)

            for j in range(1, J):
                upv = [state.tile([128, 2, BH], bf16, tag=f"upv{t}", name=f"upv{t}") for t in range(2)]
                last = j == J - 1
                for t in range(2):
                    nc.vector.tensor_tensor(
                        out=upv[t][:], in0=c01[t][:],
                        in1=pq[t][:, None, :].broadcast_to([128, 2, BH]), op=mult)
                    # final add emits the bf16 copy the leaf matmuls consume
                    nc.gpsimd.tensor_add(
                        (sacc_bf if last else sacc)[t][:], sacc[t][:], upv[t][:])
                if last:
                    break
                pq = [psum.tile([128, BH], f32, tag=f"pq{mt}", name=f"pq{mt}") for mt in range(2)]
                for mt in range(2):
                    ms = slice(mt * 128, (mt + 1) * 128)
                    ms2 = slice(512 + mt * 128, 512 + (mt + 1) * 128)
                    nc.tensor.matmul(pq[mt][:], elr[0][:, ms], upv[0][:, 0, :],
                                     start=True, stop=False)
                    nc.tensor.matmul(pq[mt][:], elr[0][:, ms2], upv[0][:, 1, :],
                                     start=False, stop=False)
                    nc.tensor.matmul(pq[mt][:], elr[1][:, ms], upv[1][:, 0, :],
                                     start=False, stop=False)
                    nc.tensor.matmul(pq[mt][:], elr[1][:, ms2], upv[1][:, 1, :],
                                     start=False, stop=True)

            # ---- leaf block (once) ----
            pleaf = psum_acc.tile([BH, 256], f32, tag="pl", name="pl")
            nc.tensor.matmul(pleaf[:], sacc_bf[0][:, 0, :], elr[0][:, 256:512],
                             start=True, stop=False)
            nc.tensor.matmul(pleaf[:], sacc_bf[0][:, 1, :], elr[0][:, 768:1024],
                             start=False, stop=False)
            nc.tensor.matmul(pleaf[:], sacc_bf[1][:, 0, :], elr[1][:, 256:512],
                             start=False, stop=False)
            nc.tensor.matmul(pleaf[:], sacc_bf[1][:, 1, :], elr[1][:, 768:1024],
                             start=False, stop=True)

            # ---- output ----
            o = work.tile([BH, 256], f32, tag="o", name="o")
            nc.vector.tensor_copy(o[:], pleaf[:])
            nc.sync.dma_start(p_out[:, :], o[:])

    nc.finalize()
    return nc


def _get_program():
    if "nc" not in _CACHE:
        _CACHE["nc"] = _build_program()
    return _CACHE["nc"]


def _prep_inputs(x, W, bias, M_left, M_right):
    """Host-side shard + layout prep. Core c -> graph c//2, batch half c%2."""
    in_maps = []
    m2_g, wt_g, bias_g = [], [], []
    for g in range(G):
        m2 = np.zeros((256, 1024), np.float32)
        tl = M_left[g].T  # (255, 511): src-major
        tr = M_right[g].T
        for base, src in ((0, tl), (512, tr)):
            m2[0:255, base:base + 255] = src[:, 0:255]
            m2[0:255, base + 256:base + 512] = src[:, 255:511]
            m2[0:255, base + 255] = NEG
        m2_g.append(m2.astype(BF16))
        wt = np.zeros((512, 256), np.float32)
        wt[:, 0:255] = W[g].T
        wt_g.append(wt)
        bp = np.zeros((256, 1), np.float32)
        bp[0:255, 0] = -bias[g]
        bias_g.append(bp)
    xt_h = [np.ascontiguousarray(x[h * BH:(h + 1) * BH].T) for h in range(2)]
    for c in range(NCORES):
        g, h = c // 2, c % 2
        wxc = np.concatenate([wt_g[g], xt_h[h]], axis=1)  # (512, 320)
        wxp = np.ascontiguousarray(
            wxc.reshape(4, 128, 320).transpose(1, 0, 2).reshape(128, 1280)
        ).astype(BF16)
        in_maps.append({
            "m2": m2_g[g], "wxp": wxp, "biasp": bias_g[g],
        })
    return in_maps


def _assemble(results):
    eps = np.float32(1e-5)
    ret = np.empty((B, L, G), np.float32)
    for c in range(NCORES):
        g, h = c // 2, c % 2
        ret[h * BH:(h + 1) * BH, :, g] = results[c]["out"]
    ret = np.where(ret > 0.0, ret, eps)
    ret = np.where(ret < 1.0, ret, np.float32(1.0) - eps)
    return ret.astype(np.float32)


def run_on_device(in_maps, trace=False, **kw):
    from concourse.bass_utils import run_bass_kernel_spmd
    nc = _get_program()
    return run_bass_kernel_spmd(nc, in_maps, list(range(NCORES)), trace=trace, **kw)


def kernel(x, W, bias, M_left, M_right):
    in_maps = _prep_inputs(
        np.asarray(x, np.float32), np.asarray(W, np.float32),
        np.asarray(bias, np.float32), np.asarray(M_left, np.float32),
        np.asarray(M_right, np.float32),
    )
    res = run_on_device(in_maps)
    return _assemble(res.results)


# revision 14
# speedup vs baseline: 1.3587x; 1.0419x over previous
"""Trainium2 Bass kernel for nn_Graphs (soft decision-graph probability propagation).

Reference math (G=4 graphs, B=128 batch, N=255 internal nodes, L=256 leaves,
F=512 features, J=8 jumps):
  b  = sigmoid(x @ W_g^T + bias_g)                  (per graph: B x N)
  M0 = softmax(M_left, axis=dest), M1 = softmax(M_right, axis=dest)
  q  = [b*(M1-M0)+M0 | leaf-identity]               (per (g,batch): 511x511)
  prob <- q @ prob, J times, starting from e0; return leaf probs.

Restructure (v2, all-bf16 datapath):
  - q never materialized. With u = prob[internal], one jump is
      u' = E0 @ (r0*(1-b)*u) + E1 @ (r1*b*u)
    where E0/E1 are raw exp(M^T) tiles (bf16) and the softmax denominators
    r0/r1 are folded into the per-(node,batch) coefficients c0/c1.
  - Leaf rows only accumulate, and c0/c1 are jump-invariant, so the leaf
    block is hoisted out of the loop entirely:
      w = E0_leaf @ (sum_j c0*u_j) + E1_leaf @ (sum_j c1*u_j)
    The running sums (sacc) are maintained by gpsimd adds in the shadow of
    the PE jump stream; 4 leaf matmuls run once at the end.
  - Jump 0 is an outer product (u_0 = e0): 4 contract-dim-1 matmuls reading
    row 0 of E0/E1 against row 0 of the coefficients.
  - exp is one fused 1024-col ACT op per src tile (both matrices at once)
    with accum_out giving the combined row sum; a DVE half-reduce splits it
    into the two softmax denominators (r1 = recip(s01 - s_el)).
  - PE warm-up (HAM un-throttle) runs first and is chained INTO the real
    dependency graph (zj = 0*pwarm feeds the c01 coefficient ops, and two
    warm matmuls WAW-target the b-matmul psum), so the scheduler cannot
    push it to the end of the program (which is what happened in v1).

Sharding: 8 cores = (graph g = core//2) x (batch half h = core%2, 64 rows).
No cross-core communication. Host pre-transposes/pads/casts to bf16:
  - m2 (256,1024) bf16: M^T with source node on partitions; cols [0:512] =
    left matrix, [512:1024] = right; each 512 block = [internal 255 | NEG |
    leaf 256] (NEG pad -> exp = 0).
  - wxp (128,1280) bf16: per F-tile k, cols [320k:320k+256] = W_g^T block,
    [320k+256:320k+320] = x_half^T block.
  - biasp (128,2) f32: +bias/2 node-tiled (device computes b via
    tanh(0.5*logit + bias/2), same ACT table set as exp).
Output per core: (64,256) f32 leaf-major; host assembles to (B,L,G) and
applies the reference interval clamp.
"""

import numpy as np
import ml_dtypes

G, B, N, L, F, J = 4, 128, 255, 256, 512, 8
BH = B // 2  # 64 batch rows per core
NCORES = 8
NEG = np.float32(-1e4)
BF16 = ml_dtypes.bfloat16

_CACHE = {}


def _build_program():
    import concourse.mybir as mybir
    from concourse import bacc
    from concourse.tile import TileContext

    f32 = mybir.dt.float32
    bf16 = mybir.dt.bfloat16
    AF = mybir.ActivationFunctionType
    AX = mybir.AxisListType
    mult = mybir.AluOpType.mult
    add = mybir.AluOpType.add

    nc = bacc.Bacc(None)
    p_m2 = nc.declare_dram_parameter("m2", [256, 1024], bf16, isOutput=False)
    p_wx = nc.declare_dram_parameter("wxp", [128, 1280], bf16, isOutput=False)
    p_bias = nc.declare_dram_parameter("biasp", [128, 2], f32, isOutput=False)
    p_out = nc.declare_dram_parameter("out", [BH, 256], f32, isOutput=True)

    with TileContext(nc) as tc:
        with (
            tc.tile_pool(name="consts", bufs=1) as consts,
            tc.tile_pool(name="work", bufs=2) as work,
            tc.tile_pool(name="state", bufs=3) as state,
            tc.tile_pool(name="psum", bufs=2, space="PSUM") as psum,
            tc.tile_pool(name="psum_acc", bufs=1, space="PSUM") as psum_acc,
        ):
            # ---- DMA issue (first: these gate everything) ----
            # Each DMA trigger occupies its issuing engine ~0.65us, and each
            # transfer's completion semaphore lands ~1-1.5us after the data
            # (HBM receipt round-trip), so the 512KB m2 matrix goes as four
            # 128KB piece so exp of piece i overlaps the transfer+receipt of
            # piece i+1. t=0 pieces on HWDGE (sync), t=1 pieces + wxp + bias
            # on SWDGE (gpsimd) so the two trigger streams run in parallel.
            ch = [consts.tile([128, 1024], bf16, tag=f"ch{t}", name=f"ch{t}") for t in range(2)]
            for half in range(2):
                cs = slice(half * 512, (half + 1) * 512)
                nc.sync.dma_start(ch[0][:, cs], p_m2[0:128, cs])
            wx = consts.tile([128, 1280], bf16, tag="wx", name="wx")
            nc.gpsimd.dma_start(wx[:], p_wx[:, :])
            for half in range(2):
                cs = slice(half * 512, (half + 1) * 512)
                nc.gpsimd.dma_start(ch[1][:, cs], p_m2[128:256, cs])
            bias = consts.tile([128, 2], f32, tag="bias", name="bias")
            nc.gpsimd.dma_start(bias[:], p_bias[:, :])

            # ---- PE warm-up (HAM un-throttle) ----
            # The jump loop's cadence includes the 8-matmul group latency,
            # which halves once the PE HAM un-throttles (needs ~3.4us of
            # sustained PE activity). Ten N=256 matmuls fill the otherwise
            # idle pre-loop PE window. The chain wsc/rsc memsets ->
            # pleaf-warms -> zw -> rsc-touch -> pq-warms -> (WAW) jump-0
            # anchors the stream in the real graph so the scheduler cannot
            # float it to the end of the program (v1's dead-code bug).
            wsc = consts.tile([128, 128], bf16, tag="wsc", name="wsc")
            rsc = consts.tile([128, 256], bf16, tag="rsc", name="rsc")
            nc.vector.memset(wsc[:], 0.0)
            nc.vector.memset(rsc[:], 0.0)
            pleaf = psum_acc.tile([BH, 256], f32, tag="pl", name="pl")
            pb = psum.tile([128, 2, BH], f32, tag="pb", name="pb")
            for _ in range(10):
                nc.tensor.matmul(pleaf[:], wsc[:, 0:BH], rsc[:], start=True, stop=True)
            zw = work.tile([BH, 1], f32, tag="zw", name="zw")
            nc.vector.tensor_scalar_mul(zw[:], pleaf[:, 0:1], 0.0)
            nc.vector.tensor_scalar_mul(rsc[0:1, 0:1], zw[0:1, :], 0.0)
            pq = [psum.tile([128, BH], f32, tag=f"pq{mt}", name=f"pq{mt}") for mt in range(2)]
            for mt in range(2):
                nc.tensor.matmul(pq[mt][:], wsc[:], rsc[:, 0:BH], start=True, stop=True)

            # ---- b = sigmoid(W @ x^T + bias) ----
            for mh in range(2):
                for k in range(4):
                    nc.tensor.matmul(
                        pb[:, mh, :],
                        wx[:, k * 320 + mh * 128:k * 320 + (mh + 1) * 128],
                        wx[:, k * 320 + 256:k * 320 + 320],
                        start=(k == 0), stop=(k == 3),
                    )

            # ---- exp of both matrices ----
            # Four 512-col slice-ops (ACT queue: el0, er0, el1, er1, eb0, eb1)
            # so each starts as soon as its chunk lands and each accum_out
            # yields that matrix-half's softmax row sum directly.
            elr = [consts.tile([128, 1024], bf16, tag=f"elr{t}", name=f"elr{t}") for t in range(2)]
            psl = [work.tile([128, 1], f32, tag=f"psl{t}", name=f"psl{t}") for t in range(2)]
            psr = [work.tile([128, 1], f32, tag=f"psr{t}", name=f"psr{t}") for t in range(2)]
            th = [work.tile([128, BH], f32, tag=f"th{mh}", name=f"th{mh}") for mh in range(2)]
            for t in range(2):
                nc.scalar.activation(elr[t][:, 0:512], ch[t][:, 0:512], AF.Exp,
                                     accum_out=psl[t][:])
                nc.scalar.activation(elr[t][:, 512:1024], ch[t][:, 512:1024], AF.Exp,
                                     accum_out=psr[t][:])
            # b via tanh (same ACT table set as exp -> no second table load):
            # th = tanh((logit + bias)/2), so b = (1+th)/2, 1-b = (1-th)/2.
            for mh in range(2):
                nc.scalar.activation(th[mh][:], pb[:, mh, :], AF.Tanh,
                                     bias=bias[:, mh:mh + 1], scale=0.5)

            # ---- softmax denominators + c01 coefficients (DVE) ----
            # c01[t][:,0,:] = r0*(1-b) = (1-th)/2 * r0
            # c01[t][:,1,:] = r1*b     = (1+th) * (r1/2)
            # with r0 = 1/psl, r1 = 1/psr from the exp accumulators. All ops
            # are single-source tensor_scalar (fast DVE mode), per-partition
            # scalars ride the AP operand.
            c01 = [consts.tile([128, 2, BH], f32, tag=f"c01{t}", name=f"c01{t}") for t in range(2)]
            c01r0 = state.tile([1, 2, BH], bf16, tag="c01r0", name="c01r0")
            for t in range(2):
                r0 = work.tile([128, 1], f32, tag=f"r0_{t}", name=f"r0_{t}")
                nc.vector.reciprocal(r0[:], psl[t][:])
                psr2 = work.tile([128, 1], f32, tag="psr2", name=f"psr2_{t}")
                nc.vector.tensor_scalar_mul(psr2[:], psr[t][:], 2.0)
                r1h = work.tile([128, 1], f32, tag=f"r1h_{t}", name=f"r1h_{t}")
                nc.vector.reciprocal(r1h[:], psr2[:])
                nc.vector.tensor_scalar(
                    c01[t][:, 1, :], th[t][:], 1.0, r1h[:], op0=add, op1=mult)
                tb = work.tile([128, BH], f32, tag="tb", name=f"tb{t}")
                nc.vector.tensor_scalar(tb[:], th[t][:], -0.5, 0.5, op0=mult, op1=add)
                nc.vector.tensor_scalar_mul(c01[t][:, 0, :], tb[:], r0[:])
                if t == 0:
                    # bf16 seed state for jump 0 (= c01 row 0, since u_0 = e0)
                    nc.vector.tensor_copy(c01r0[:], c01[0][0:1, :, :])

            # ---- leaf-sum accumulators (maintained by gpsimd) ----
            # sacc f32 through jump 6; jump 7's add writes the bf16 copy
            # directly (the cast the leaf matmuls need comes for free).
            sacc = [consts.tile([128, 2, BH], f32, tag=f"sacc{t}", name=f"sacc{t}") for t in range(2)]
            sacc_bf = [state.tile([128, 2, BH], bf16, tag=f"sbf{t}", name=f"sbf{t}") for t in range(2)]
            nc.gpsimd.memset(sacc[0][:], 0.0)
            nc.gpsimd.memset(sacc[1][:], 0.0)
            nc.gpsimd.tensor_add(sacc[0][0:1, :, :], sacc[0][0:1, :, :], c01r0[:])

            # ---- jump loop ----
            # Jump 0: u_1 = outer(E row0, c01 row0) -> 4 contract-1 matmuls.
            # Jumps 1..6: full rounds, 8 matmuls each, grouped by dest tile
            # so the next jump's DVE scale op starts after the first group.
            # Jump 7: scale+accumulate only (u_8 internal mass is unused).
            # (pq tiles were pre-allocated above as the warm-up WAW target.)
            for mt in range(2):
                ms = slice(mt * 128, (mt + 1) * 128)
                nc.tensor.matmul(pq[mt][:], elr[0][0:1, ms], c01r0[0:1, 0, :],
                                 start=True, stop=False)
                ms2 = slice(512 + mt * 128, 512 + (mt + 1) * 128)
                nc.tensor.matmul(pq[mt][:], elr[0][0:1, ms2], c01r0[0:1, 1, :],
                                 start=False, stop=True)

            for j in range(1, J):
                upv = [state.tile([128, 2, BH], bf16, tag=f"upv{t}", name=f"upv{t}") for t in range(2)]
                last = j == J - 1
                for t in range(2):
                    nc.vector.tensor_tensor(
                        out=upv[t][:], in0=c01[t][:],
                        in1=pq[t][:, None, :].broadcast_to([128, 2, BH]), op=mult)
                    # final add emits the bf16 copy the leaf matmuls consume
                    nc.gpsimd.tensor_add(
                        (sacc_bf if last else sacc)[t][:], sacc[t][:], upv[t][:])
                if last:
                    break
                pq = [psum.tile([128, BH], f32, tag=f"pq{mt}", name=f"pq{mt}") for mt in range(2)]
                for mt in range(2):
                    ms = slice(mt * 128, (mt + 1) * 128)
                    ms2 = slice(512 + mt * 128, 512 + (mt + 1) * 128)
                    nc.tensor.matmul(pq[mt][:], elr[0][:, ms], upv[0][:, 0, :],
                                     start=True, stop=False)
                    nc.tensor.matmul(pq[mt][:], elr[0][:, ms2], upv[0][:, 1, :],
                                     start=False, stop=False)
                    nc.tensor.matmul(pq[mt][:], elr[1][:, ms], upv[1][:, 0, :],
                                     start=False, stop=False)
                    nc.tensor.matmul(pq[mt][:], elr[1][:, ms2], upv[1][:, 1, :],
                                     start=False, stop=True)

            # ---- leaf block (once; pleaf pre-allocated as warm-up target) ----
            nc.tensor.matmul(pleaf[:], sacc_bf[0][:, 0, :], elr[0][:, 256:512],
                             start=True, stop=False)
            nc.tensor.matmul(pleaf[:], sacc_bf[0][:, 1, :], elr[0][:, 768:1024],
                             start=False, stop=False)
            nc.tensor.matmul(pleaf[:], sacc_bf[1][:, 0, :], elr[1][:, 256:512],
                             start=False, stop=False)
            nc.tensor.matmul(pleaf[:], sacc_bf[1][:, 1, :], elr[1][:, 768:1024],
                             start=False, stop=True)

            # ---- output ----
            o = work.tile([BH, 256], f32, tag="o", name="o")
            nc.vector.tensor_copy(o[:], pleaf[:])
            nc.sync.dma_start(p_out[:, :], o[:])

    nc.finalize()
    return nc


def _get_program():
    if "nc" not in _CACHE:
        _CACHE["nc"] = _build_program()
    return _CACHE["nc"]


def _prep_inputs(x, W, bias, M_left, M_right):
    """Host-side shard + layout prep. Core c -> graph c//2, batch half c%2."""
    in_maps = []
    m2_g, wt_g, bias_g = [], [], []
    for g in range(G):
        m2 = np.zeros((256, 1024), np.float32)
        tl = M_left[g].T  # (255, 511): src-major
        tr = M_right[g].T
        for base, src in ((0, tl), (512, tr)):
            m2[0:255, base:base + 255] = src[:, 0:255]
            m2[0:255, base + 256:base + 512] = src[:, 255:511]
            m2[0:255, base + 255] = NEG
        m2_g.append(m2.astype(BF16))
        wt = np.zeros((512, 256), np.float32)
        wt[:, 0:255] = W[g].T
        wt_g.append(wt)
        bp = np.zeros((256,), np.float32)
        bp[0:255] = bias[g] * 0.5
        bias_g.append(np.ascontiguousarray(bp.reshape(2, 128).T))  # (128, 2)
    xt_h = [np.ascontiguousarray(x[h * BH:(h + 1) * BH].T) for h in range(2)]
    for c in range(NCORES):
        g, h = c // 2, c % 2
        wxc = np.concatenate([wt_g[g], xt_h[h]], axis=1)  # (512, 320)
        wxp = np.ascontiguousarray(
            wxc.reshape(4, 128, 320).transpose(1, 0, 2).reshape(128, 1280)
        ).astype(BF16)
        in_maps.append({
            "m2": m2_g[g], "wxp": wxp, "biasp": bias_g[g],
        })
    return in_maps


def _assemble(results):
    eps = np.float32(1e-5)
    ret = np.empty((B, L, G), np.float32)
    for c in range(NCORES):
        g, h = c // 2, c % 2
        ret[h * BH:(h + 1) * BH, :, g] = results[c]["out"]
    ret = np.where(ret > 0.0, ret, eps)
    ret = np.where(ret < 1.0, ret, np.float32(1.0) - eps)
    return ret.astype(np.float32)


def run_on_device(in_maps, trace=False, **kw):
    from concourse.bass_utils import run_bass_kernel_spmd
    nc = _get_program()
    return run_bass_kernel_spmd(nc, in_maps, list(range(NCORES)), trace=trace, **kw)


def kernel(x, W, bias, M_left, M_right):
    in_maps = _prep_inputs(
        np.asarray(x, np.float32), np.asarray(W, np.float32),
        np.asarray(bias, np.float32), np.asarray(M_left, np.float32),
        np.asarray(M_right, np.float32),
    )
    res = run_on_device(in_maps)
    return _assemble(res.results)
